# revision 5
# baseline (speedup 1.0000x reference)
"""Trainium2 Bass kernel for nn_LogicConv3d (differentiable logic-gate 3D conv).

Strategy
--------
The reference's big gather `x.reshape(B,-1)[:, lin]` reads shifted 30x30x30
windows of the (C,32,32,32) volume: coords lie in [0,3), so each (j,k,s) leaf
operand is one of 81 shifted slices (c,dh,dw,dd).  Each tree node is a
bilinear blend  out = c0 + ca*a + cb*b + cab*a*b  whose coefficients come from
softmax(w)@GATES — tiny, computed on host.  Constants are folded into parents
(the bilinear form is closed under constant shifts of its inputs).

Sharding: kernels K=32 split 4-per-core across 8 cores (batch packed into the
partition/flat-position dim).  Per-core differences are pure DATA, so ONE SPMD
program runs on all 8 cores via run_bass_kernel_spmd.

Device op mix (v2): scalar_tensor_tensor has NO fast DVE mode (1060ns per
(128,844) fp16 tile) while tensor_scalar hits 4x (~350ns) and tensor_tensor
2x (~580ns).  Per node:
    u = TS(b, cab, ca)        # cab*b + ca     ACT (scalar engine) or DVE
    t = TT_mult(a, u)         # a*u            DVE
    o = STT(b, cb, t)         # cb*b + t       GPSIMD or DVE
Root nodes instead use  v = TS(b, cb, gamma); o = TT_add(t, v)  which folds
the root constant and writes the fp16 output tile directly (host casts to
fp32).  The split fractions below balance ACT/DVE/GPSIMD at ~118us/core.

DMA: leaf operands are host-pre-gathered into per-kernel-chunk contiguous
arrays (8 leaves x 844 positions per chunk) so inputs arrive in 32 large
~1.7MB DMAs instead of 128 small ones.
"""
import numpy as np

# ---- problem constants (hardcoded per contest contract) ----
B, C, H, W, D = 4, 3, 32, 32, 32
K, S = 32, 16
OH = OW = OD = 30
P = OH * OW * OD            # 27000
BP = B * P                  # 108000
NPART = 128
FREE = (BP + NPART - 1) // NPART   # 844
PADBP = NPART * FREE        # 108032
NCORES = 8
KLOC = K // NCORES          # 4
TEMP = 1.0
NLEV = 5
NODES_PER_K = 31            # 16+8+4+2+1
CHUNK = 8                   # leaves per input DMA chunk
NCHUNK = S // CHUNK         # 2 per kernel per operand
CFREE = CHUNK * FREE        # 6752
NCOLS = KLOC * (30 * 3 + 4)  # 376 coef columns (30 non-root x3 + root x4)

GATES = np.array([[(g >> t) & 1 for t in range(4)] for g in range(16)],
                 dtype=np.float64)

# engine assignment knobs (LP-balanced: ACT ~143us, DVE ~144us, GPS ~142us)
# GPSIMD supports only tensor_tensor, so GPS nodes ("way 2") compute
# o = TT_add(t, v) with an extra v = cb*b tensor_scalar; DVE nodes ("way 1")
# compute o = STT(b, cb, t) directly.
TS_DVE_RES = (0, 1, 2)   # TS op -> DVE when ts_idx % 8 in RES, else ACT
TS_DVE_MODB = 8
GPS_MOD = 3              # o-op -> DVE STT when o_idx % GPS_MOD == 0, else GPS
USE_GPS = True


# ----------------------------------------------------------------- host math
def _lut_coeffs(w):
    """w: (nodes,K,16) -> c0, ca, cb, cab each (nodes,K) float64."""
    w = w.astype(np.float64)
    e = np.exp((w - w.max(-1, keepdims=True)) / TEMP)
    p = e / e.sum(-1, keepdims=True)
    l = p @ GATES
    l0, l1, l2, l3 = l[..., 0], l[..., 1], l[..., 2], l[..., 3]
    return l0, l2 - l0, l1 - l0, l0 - l1 - l2 + l3


def _fold_coeffs(ws):
    """Fold per-node constants into parents.  Returns (folded, root_const):
    folded[lev] = (ca2, cb2, cab) each (nodes,K); root_const (K,)."""
    folded = []
    gamma = None
    for lev, w in enumerate(ws):
        c0, ca, cb, cab = _lut_coeffs(w)
        if lev == 0:
            gA = np.zeros_like(c0)
            gB = np.zeros_like(c0)
        else:
            gA = gamma[0::2]
            gB = gamma[1::2]
        folded.append((ca + cab * gB, cb + cab * gA, cab))
        gamma = c0 + ca * gA + cb * gB + cab * gA * gB
    return folded, gamma[0]


def _coef_cols(k, folded, root_const):
    """Per-kernel coef column values, in device op order."""
    cols = []
    for lev in range(NLEV):
        ca2, cb2, cab = folded[lev]
        n = ca2.shape[0]
        for i in range(n):
            if lev == NLEV - 1:
                cols += [cab[i, k], ca2[i, k], cb2[i, k], root_const[k]]
            else:
                cols += [cab[i, k], ca2[i, k], cb2[i, k]]
    return cols


def _prep_inputs(x, kc, ws):
    """Build per-core in_maps (numpy)."""
    # 81 shifted windows, flattened positions (b,oh,ow,od), fp16, padded
    X81 = np.empty((3, 3, 3, 3, B, OH, OW, OD), np.float32)
    for c in range(3):
        for dh in range(3):
            for dw in range(3):
                for dd in range(3):
                    X81[c, dh, dw, dd] = x[:, c, dh:dh + 30, dw:dw + 30, dd:dd + 30]
    X81f = np.zeros((81, NPART, FREE), np.float16)
    X81f.reshape(81, PADBP)[:, :BP] = X81.reshape(81, BP).astype(np.float16)

    h_, w_, d_, c_ = kc[..., 0], kc[..., 1], kc[..., 2], kc[..., 3]
    sl = ((c_ * 3 + h_) * 3 + w_) * 3 + d_          # (2,K,S)

    folded, root_const = _fold_coeffs(ws)

    in_maps = []
    for core in range(NCORES):
        ks = range(core * KLOC, (core + 1) * KLOC)
        a_in = np.empty((KLOC * NCHUNK, NPART, CFREE), np.float16)
        b_in = np.empty((KLOC * NCHUNK, NPART, CFREE), np.float16)
        colv = []
        for kk, k in enumerate(ks):
            for c in range(NCHUNK):
                idx0 = sl[0, k, c * CHUNK:(c + 1) * CHUNK]
                idx1 = sl[1, k, c * CHUNK:(c + 1) * CHUNK]
                a_in[kk * NCHUNK + c] = \
                    X81f[idx0].transpose(1, 0, 2).reshape(NPART, CFREE)
                b_in[kk * NCHUNK + c] = \
                    X81f[idx1].transpose(1, 0, 2).reshape(NPART, CFREE)
            colv += _coef_cols(k, folded, root_const)
        assert len(colv) == NCOLS
        coef = np.broadcast_to(
            np.asarray(colv, np.float32), (NPART, NCOLS)).copy()
        in_maps.append({"a_in": a_in, "b_in": b_in, "coef": coef})
    return in_maps


# ------------------------------------------------------------ device program
def _build_program():
    import concourse.bass as bass
    import concourse.bacc as bacc
    import concourse.mybir as mybir
    from concourse.tile import TileContext

    f16 = mybir.dt.float16
    f32 = mybir.dt.float32
    Alu = mybir.AluOpType
    Act = mybir.ActivationFunctionType

    nc = bacc.Bacc()
    a_in = nc.declare_dram_parameter("a_in", [KLOC * NCHUNK, NPART, CFREE],
                                     f16, isOutput=False)
    b_in = nc.declare_dram_parameter("b_in", [KLOC * NCHUNK, NPART, CFREE],
                                     f16, isOutput=False)
    coef = nc.declare_dram_parameter("coef", [NPART, NCOLS], f32,
                                     isOutput=False)
    out = nc.declare_dram_parameter("out", [KLOC, NPART, FREE], f16,
                                    isOutput=True)

    ts_idx = 0
    o_idx = 0
    col = 0

    with TileContext(nc) as tc:
        with (
            tc.tile_pool(name="cpool", bufs=1) as cpool,
            tc.tile_pool(name="apool", bufs=4) as apool,
            tc.tile_pool(name="bpool", bufs=4) as bpool,
            tc.tile_pool(name="upool", bufs=5) as upool,
            tc.tile_pool(name="tpool", bufs=5) as tpool,
            tc.tile_pool(name="lpool", bufs=2) as lpool,
            tc.tile_pool(name="opool", bufs=3) as opool,
        ):
            coef_sb = cpool.tile([NPART, NCOLS], f32)
            nc.sync.dma_start(out=coef_sb[:], in_=coef[:])

            def ts_op(dst, src, scale_ap, bias_ap):
                nonlocal ts_idx
                if ts_idx % TS_DVE_MODB in TS_DVE_RES:
                    if bias_ap is None:
                        nc.vector.tensor_scalar(dst, src, scale_ap, None,
                                                Alu.mult)
                    else:
                        nc.vector.tensor_scalar(dst, src, scale_ap, bias_ap,
                                                Alu.mult, Alu.add)
                else:
                    nc.scalar.activation(dst, src, Act.Identity,
                                         bias=bias_ap if bias_ap is not None
                                         else 0.0,
                                         scale=scale_ap)
                ts_idx += 1

            def node_eval(a_ap, b_ap, lev, is_root=False, out_t=None):
                nonlocal o_idx, col
                cab_ap = coef_sb[:, col:col + 1]
                ca_ap = coef_sb[:, col + 1:col + 2]
                cb_ap = coef_sb[:, col + 2:col + 3]
                u = upool.tile([NPART, FREE], f16, tag="u", name=f"u{col}")
                ts_op(u[:], b_ap, cab_ap, ca_ap)
                t = tpool.tile([NPART, FREE], f16, tag="t", name=f"t{col}")
                nc.vector.tensor_tensor(out=t[:], in0=a_ap, in1=u[:],
                                        op=Alu.mult)
                if is_root:
                    g_ap = coef_sb[:, col + 3:col + 4]
                    v = upool.tile([NPART, FREE], f16, tag="u",
                                   name=f"v{col}")
                    ts_op(v[:], b_ap, cb_ap, g_ap)
                    nc.vector.tensor_tensor(out=out_t[:], in0=t[:], in1=v[:],
                                            op=Alu.add)
                    col += 4
                    return out_t
                o = lpool.tile([NPART, FREE], f16, tag=f"o{lev}",
                               name=f"o{col}", bufs=(18 >> lev) + 2)
                if USE_GPS and (o_idx % GPS_MOD) != 0:
                    v = tpool.tile([NPART, FREE], f16, tag="t",
                                   name=f"vv{col}")
                    ts_op(v[:], b_ap, cb_ap, None)
                    nc.gpsimd.tensor_tensor(out=o[:], in0=t[:], in1=v[:],
                                            op=Alu.add)
                else:
                    nc.vector.scalar_tensor_tensor(
                        o[:], b_ap, cb_ap, t[:], Alu.mult, Alu.add)
                o_idx += 1
                col += 3
                return o

            for kk in range(KLOC):
                ach = []
                bch = []
                for c in range(NCHUNK):
                    at = apool.tile([NPART, CFREE], f16, tag="a",
                                    name=f"a{kk}_{c}")
                    nc.sync.dma_start(out=at[:], in_=a_in[kk * NCHUNK + c])
                    bt = bpool.tile([NPART, CFREE], f16, tag="b",
                                    name=f"b{kk}_{c}")
                    nc.sync.dma_start(out=bt[:], in_=b_in[kk * NCHUNK + c])
                    ach.append(at)
                    bch.append(bt)
                cur = []
                for s in range(S):
                    a_ap = ach[s // CHUNK][:, (s % CHUNK) * FREE:
                                           (s % CHUNK + 1) * FREE]
                    b_ap = bch[s // CHUNK][:, (s % CHUNK) * FREE:
                                           (s % CHUNK + 1) * FREE]
                    cur.append(node_eval(a_ap, b_ap, 0))
                for lev in range(1, NLEV):
                    is_root = lev == NLEV - 1
                    out_t = None
                    if is_root:
                        out_t = opool.tile([NPART, FREE], f16, tag="out",
                                           name=f"ot{kk}")
                    nxt = []
                    for i in range(len(cur) // 2):
                        nxt.append(node_eval(
                            cur[2 * i][:], cur[2 * i + 1][:], lev,
                            is_root=is_root, out_t=out_t))
                    cur = nxt
                nc.sync.dma_start(out=out[kk], in_=cur[0][:])
    nc.compile()
    return nc


_PROGRAM = None


def kernel(**inputs):
    global _PROGRAM
    x = np.asarray(inputs["x"], dtype=np.float32)
    kc = np.asarray(inputs["kernel_coords"])
    ws = [np.asarray(inputs[f"w{i}"]) for i in range(5)]

    in_maps = _prep_inputs(x, kc, ws)

    from concourse.bass_utils import run_bass_kernel_spmd
    if _PROGRAM is None:
        _PROGRAM = _build_program()
    res = run_bass_kernel_spmd(_PROGRAM, in_maps, list(range(NCORES)))
    results = res.results

    full = np.empty((K, PADBP), np.float32)
    for core in range(NCORES):
        o = results[core]["out"].reshape(KLOC, PADBP)
        full[core * KLOC:(core + 1) * KLOC] = o
    out = full[:, :BP].reshape(K, B, OH, OW, OD).transpose(1, 0, 2, 3, 4)
    return np.ascontiguousarray(out)


# revision 8
# speedup vs baseline: 1.0306x; 1.0306x over previous
"""Trainium2 Bass kernel for nn_LogicConv3d (differentiable logic-gate 3D conv).

Strategy
--------
The reference's big gather `x.reshape(B,-1)[:, lin]` reads shifted 30x30x30
windows of the (C,32,32,32) volume: coords lie in [0,3), so each (j,k,s) leaf
operand is one of 81 shifted slices (c,dh,dw,dd).  Each tree node is a
bilinear blend  out = c0 + ca*a + cb*b + cab*a*b  whose coefficients come from
softmax(w)@GATES — tiny, computed on host.  Constants are folded into parents
(the bilinear form is closed under constant shifts of its inputs).

Sharding: kernels K=32 split 4-per-core across 8 cores (batch packed into the
partition/flat-position dim).  Per-core differences are pure DATA, so ONE SPMD
program runs on all 8 cores via run_bass_kernel_spmd.

Device op mix (v2): scalar_tensor_tensor has NO fast DVE mode (1060ns per
(128,844) fp16 tile) while tensor_scalar hits 4x (~350ns) and tensor_tensor
2x (~580ns).  Per node:
    u = TS(b, cab, ca)        # cab*b + ca     ACT (scalar engine) or DVE
    t = TT_mult(a, u)         # a*u            DVE
    o = STT(b, cb, t)         # cb*b + t       GPSIMD or DVE
Root nodes instead use  v = TS(b, cb, gamma); o = TT_add(t, v)  which folds
the root constant and writes the fp16 output tile directly (host casts to
fp32).  The split fractions below balance ACT/DVE/GPSIMD at ~118us/core.

DMA: leaf operands are host-pre-gathered into per-kernel-chunk contiguous
arrays (8 leaves x 844 positions per chunk) so inputs arrive in 32 large
~1.7MB DMAs instead of 128 small ones.
"""
import numpy as np

# ---- problem constants (hardcoded per contest contract) ----
B, C, H, W, D = 4, 3, 32, 32, 32
K, S = 32, 16
OH = OW = OD = 30
P = OH * OW * OD            # 27000
BP = B * P                  # 108000
NPART = 128
FREE = (BP + NPART - 1) // NPART   # 844
PADBP = NPART * FREE        # 108032
NCORES = 8
KLOC = K // NCORES          # 4
TEMP = 1.0
NLEV = 5
NODES_PER_K = 31            # 16+8+4+2+1
CHUNK = 4                   # leaves per input DMA chunk
NCHUNK = S // CHUNK         # 4 per kernel per operand
CFREE = CHUNK * FREE        # 3376
NCOLS = KLOC * (30 * 3 + 4)  # 376 coef columns (30 non-root x3 + root x4)
LEV_OFF = [0, 48, 72, 84, 90]  # per-kernel coef column offset by level

GATES = np.array([[(g >> t) & 1 for t in range(4)] for g in range(16)],
                 dtype=np.float64)

# engine assignment knobs (LP-balanced: DVE/ACT/GPS each ~155us pure)
# GPSIMD supports only tensor_tensor, so GPS nodes ("way 2") compute
# o = TT_add(t, v) with an extra v = cb*b tensor_scalar; DVE nodes ("way 1")
# compute o = STT(b, cb, t) directly.
TS_DVE_RES = (2,)        # TS op -> DVE when ts_idx % TS_DVE_MODB in RES
TS_DVE_MODB = 3
WAY1_RES = (0, 1)        # o-op -> DVE STT when o_idx % WAY1_MODB in RES
WAY1_MODB = 5
USE_GPS = True


# ----------------------------------------------------------------- host math
def _lut_coeffs(w):
    """w: (nodes,K,16) -> c0, ca, cb, cab each (nodes,K) float64."""
    w = w.astype(np.float64)
    e = np.exp((w - w.max(-1, keepdims=True)) / TEMP)
    p = e / e.sum(-1, keepdims=True)
    l = p @ GATES
    l0, l1, l2, l3 = l[..., 0], l[..., 1], l[..., 2], l[..., 3]
    return l0, l2 - l0, l1 - l0, l0 - l1 - l2 + l3


def _fold_coeffs(ws):
    """Fold per-node constants into parents.  Returns (folded, root_const):
    folded[lev] = (ca2, cb2, cab) each (nodes,K); root_const (K,)."""
    folded = []
    gamma = None
    for lev, w in enumerate(ws):
        c0, ca, cb, cab = _lut_coeffs(w)
        if lev == 0:
            gA = np.zeros_like(c0)
            gB = np.zeros_like(c0)
        else:
            gA = gamma[0::2]
            gB = gamma[1::2]
        folded.append((ca + cab * gB, cb + cab * gA, cab))
        gamma = c0 + ca * gA + cb * gB + cab * gA * gB
    return folded, gamma[0]


def _coef_cols(k, folded, root_const):
    """Per-kernel coef column values, in device op order."""
    cols = []
    for lev in range(NLEV):
        ca2, cb2, cab = folded[lev]
        n = ca2.shape[0]
        for i in range(n):
            if lev == NLEV - 1:
                cols += [cab[i, k], ca2[i, k], cb2[i, k], root_const[k]]
            else:
                cols += [cab[i, k], ca2[i, k], cb2[i, k]]
    return cols


def _prep_inputs(x, kc, ws):
    """Build per-core in_maps (numpy)."""
    # 81 shifted windows, flattened positions (b,oh,ow,od), fp16, padded
    X81 = np.empty((3, 3, 3, 3, B, OH, OW, OD), np.float32)
    for c in range(3):
        for dh in range(3):
            for dw in range(3):
                for dd in range(3):
                    X81[c, dh, dw, dd] = x[:, c, dh:dh + 30, dw:dw + 30, dd:dd + 30]
    X81f = np.zeros((81, NPART, FREE), np.float16)
    X81f.reshape(81, PADBP)[:, :BP] = X81.reshape(81, BP).astype(np.float16)

    h_, w_, d_, c_ = kc[..., 0], kc[..., 1], kc[..., 2], kc[..., 3]
    sl = ((c_ * 3 + h_) * 3 + w_) * 3 + d_          # (2,K,S)

    folded, root_const = _fold_coeffs(ws)

    in_maps = []
    for core in range(NCORES):
        ks = range(core * KLOC, (core + 1) * KLOC)
        a_in = np.empty((KLOC * NCHUNK, NPART, CFREE), np.float16)
        b_in = np.empty((KLOC * NCHUNK, NPART, CFREE), np.float16)
        colv = []
        for kk, k in enumerate(ks):
            for c in range(NCHUNK):
                idx0 = sl[0, k, c * CHUNK:(c + 1) * CHUNK]
                idx1 = sl[1, k, c * CHUNK:(c + 1) * CHUNK]
                a_in[kk * NCHUNK + c] = \
                    X81f[idx0].transpose(1, 0, 2).reshape(NPART, CFREE)
                b_in[kk * NCHUNK + c] = \
                    X81f[idx1].transpose(1, 0, 2).reshape(NPART, CFREE)
            colv += _coef_cols(k, folded, root_const)
        assert len(colv) == NCOLS
        coef = np.broadcast_to(
            np.asarray(colv, np.float32), (NPART, NCOLS)).copy()
        in_maps.append({"a_in": a_in, "b_in": b_in, "coef": coef})
    return in_maps


# ------------------------------------------------------------ device program
def _build_program():
    import concourse.bass as bass
    import concourse.bacc as bacc
    import concourse.mybir as mybir
    from concourse.tile import TileContext

    f16 = mybir.dt.float16
    f32 = mybir.dt.float32
    Alu = mybir.AluOpType
    Act = mybir.ActivationFunctionType

    nc = bacc.Bacc()
    a_in = nc.declare_dram_parameter("a_in", [KLOC * NCHUNK, NPART, CFREE],
                                     f16, isOutput=False)
    b_in = nc.declare_dram_parameter("b_in", [KLOC * NCHUNK, NPART, CFREE],
                                     f16, isOutput=False)
    coef = nc.declare_dram_parameter("coef", [NPART, NCOLS], f32,
                                     isOutput=False)
    out = nc.declare_dram_parameter("out", [KLOC, NPART, FREE], f16,
                                    isOutput=True)

    ts_idx = 0
    o_idx = 0

    def eager_nodes():
        """Post-order (eager-reduction) node sequence for one kernel."""
        seq = []
        for s in range(S):
            seq.append((0, s))
            l, i = 0, s
            while i % 2 == 1 and l < NLEV - 1:
                l, i = l + 1, i // 2
                seq.append((l, i))
        return seq

    with TileContext(nc) as tc:
        with (
            tc.tile_pool(name="cpool", bufs=1) as cpool,
            tc.tile_pool(name="apool", bufs=9) as apool,
            tc.tile_pool(name="bpool", bufs=9) as bpool,
            tc.tile_pool(name="upool", bufs=5) as upool,
            tc.tile_pool(name="vpool", bufs=5) as vpool,
            tc.tile_pool(name="tpool", bufs=5) as tpool,
            tc.tile_pool(name="lpool", bufs=2) as lpool,
            tc.tile_pool(name="opool", bufs=3) as opool,
        ):
            coef_sb = cpool.tile([NPART, NCOLS], f32)
            nc.sync.dma_start(out=coef_sb[:], in_=coef[:])

            def ts_op(dst, src, scale_ap, bias_ap):
                nonlocal ts_idx
                if ts_idx % TS_DVE_MODB in TS_DVE_RES:
                    if bias_ap is None:
                        nc.vector.tensor_scalar(dst, src, scale_ap, None,
                                                Alu.mult)
                    else:
                        nc.vector.tensor_scalar(dst, src, scale_ap, bias_ap,
                                                Alu.mult, Alu.add)
                else:
                    nc.scalar.activation(dst, src, Act.Identity,
                                         bias=bias_ap if bias_ap is not None
                                         else 0.0,
                                         scale=scale_ap)
                ts_idx += 1

            # per-(kernel, lev, idx) state
            achunk = {}
            bchunk = {}
            otile = {}
            state = {}

            def col_of(kk, lev, i):
                return kk * 94 + LEV_OFF[lev] + (4 if lev == NLEV - 1
                                                 else 3) * i

            def inputs(kk, lev, i):
                if lev == 0:
                    c, j = i // CHUNK, i % CHUNK
                    a_ap = achunk[kk, c][:, j * FREE:(j + 1) * FREE]
                    b_ap = bchunk[kk, c][:, j * FREE:(j + 1) * FREE]
                    return a_ap, b_ap
                return (otile[kk, lev - 1, 2 * i][:],
                        otile[kk, lev - 1, 2 * i + 1][:])

            def stage_ts(kk, lev, i):
                nonlocal o_idx
                col = col_of(kk, lev, i)
                a_ap, b_ap = inputs(kk, lev, i)
                is_root = lev == NLEV - 1
                way1 = (not USE_GPS) or (o_idx % WAY1_MODB in WAY1_RES)
                if not is_root:
                    o_idx += 1
                u = upool.tile([NPART, FREE], f16, tag="u",
                               name=f"u{kk}_{lev}_{i}")
                ts_op(u[:], b_ap, coef_sb[:, col:col + 1],
                      coef_sb[:, col + 1:col + 2])
                v = None
                if is_root:
                    v = vpool.tile([NPART, FREE], f16, tag="v",
                                   name=f"v{kk}_{lev}_{i}")
                    ts_op(v[:], b_ap, coef_sb[:, col + 2:col + 3],
                          coef_sb[:, col + 3:col + 4])
                elif not way1:
                    v = vpool.tile([NPART, FREE], f16, tag="v",
                                   name=f"v{kk}_{lev}_{i}")
                    ts_op(v[:], b_ap, coef_sb[:, col + 2:col + 3], None)
                state[kk, lev, i] = (u, v, way1, a_ap, b_ap)

            def stage_tt(kk, lev, i):
                u, v, way1, a_ap, b_ap = state[kk, lev, i]
                t = tpool.tile([NPART, FREE], f16, tag="t",
                               name=f"t{kk}_{lev}_{i}")
                nc.vector.tensor_tensor(out=t[:], in0=a_ap, in1=u[:],
                                        op=Alu.mult)
                state[kk, lev, i] = (t, v, way1, a_ap, b_ap)

            def stage_o(kk, lev, i):
                t, v, way1, a_ap, b_ap = state.pop((kk, lev, i))
                col = col_of(kk, lev, i)
                is_root = lev == NLEV - 1
                if is_root:
                    ot = opool.tile([NPART, FREE], f16, tag="out",
                                    name=f"ot{kk}")
                    nc.vector.tensor_tensor(out=ot[:], in0=t[:], in1=v[:],
                                            op=Alu.add)
                    nc.sync.dma_start(out=out[kk], in_=ot[:])
                    return
                o = lpool.tile([NPART, FREE], f16, tag=f"o{lev}",
                               name=f"o{kk}_{lev}_{i}",
                               bufs=8 if lev == 0 else 6)
                if way1:
                    nc.vector.scalar_tensor_tensor(
                        o[:], b_ap, coef_sb[:, col + 2:col + 3], t[:],
                        Alu.mult, Alu.add)
                else:
                    nc.gpsimd.tensor_tensor(out=o[:], in0=t[:], in1=v[:],
                                            op=Alu.add)
                otile[kk, lev, i] = o

            for pair in range(KLOC // 2):
                kA, kB = 2 * pair, 2 * pair + 1
                for c in range(NCHUNK):
                    for kk in (kA, kB):
                        at = apool.tile([NPART, CFREE], f16, tag="a",
                                        name=f"a{kk}_{c}")
                        nc.sync.dma_start(out=at[:],
                                          in_=a_in[kk * NCHUNK + c])
                        achunk[kk, c] = at
                        bt = bpool.tile([NPART, CFREE], f16, tag="b",
                                        name=f"b{kk}_{c}")
                        nc.sync.dma_start(out=bt[:],
                                          in_=b_in[kk * NCHUNK + c])
                        bchunk[kk, c] = bt
                # interleave the two kernels' eager node streams
                nodes = []
                for na, nb in zip(eager_nodes(), eager_nodes()):
                    nodes.append((kA,) + na)
                    nodes.append((kB,) + nb)
                # software-pipelined emission: O(j-2), TT(j-1), TS(j)
                n = len(nodes)
                for j in range(n + 2):
                    if j >= 2:
                        stage_o(*nodes[j - 2])
                    if 1 <= j <= n:
                        stage_tt(*nodes[j - 1])
                    if j < n:
                        stage_ts(*nodes[j])
    nc.compile()
    return nc


_PROGRAM = None


def kernel(**inputs):
    global _PROGRAM
    x = np.asarray(inputs["x"], dtype=np.float32)
    kc = np.asarray(inputs["kernel_coords"])
    ws = [np.asarray(inputs[f"w{i}"]) for i in range(5)]

    in_maps = _prep_inputs(x, kc, ws)

    from concourse.bass_utils import run_bass_kernel_spmd
    if _PROGRAM is None:
        _PROGRAM = _build_program()
    res = run_bass_kernel_spmd(_PROGRAM, in_maps, list(range(NCORES)))
    results = res.results

    full = np.empty((K, PADBP), np.float32)
    for core in range(NCORES):
        o = results[core]["out"].reshape(KLOC, PADBP)
        full[core * KLOC:(core + 1) * KLOC] = o
    out = full[:, :BP].reshape(K, B, OH, OW, OD).transpose(1, 0, 2, 3, 4)
    return np.ascontiguousarray(out)


# revision 13
# speedup vs baseline: 1.6834x; 1.6334x over previous
"""Trainium2 Bass kernel for nn_LogicConv3d (differentiable logic-gate 3D conv).

Strategy
--------
The reference's big gather `x.reshape(B,-1)[:, lin]` reads shifted 30x30x30
windows of the (C,32,32,32) volume: coords lie in [0,3), so each (j,k,s) leaf
operand is one of 81 shifted slices (c,dh,dw,dd).  Each tree node is a
bilinear blend  out = c0 + ca*a + cb*b + cab*a*b  whose coefficients come from
softmax(w)@GATES — tiny, computed on host.  Constants are folded into parents
(the bilinear form is closed under constant shifts of its inputs).

Sharding: kernels K=32 split 4-per-core across 8 cores (batch packed into the
partition/flat-position dim).  Per-core differences are pure DATA, so ONE SPMD
program runs on all 8 cores via run_bass_kernel_spmd.

Device op mix (v2): scalar_tensor_tensor has NO fast DVE mode (1060ns per
(128,844) fp16 tile) while tensor_scalar hits 4x (~350ns) and tensor_tensor
2x (~580ns).  Per node:
    u = TS(b, cab, ca)        # cab*b + ca     ACT (scalar engine) or DVE
    t = TT_mult(a, u)         # a*u            DVE
    o = STT(b, cb, t)         # cb*b + t       GPSIMD or DVE
Root nodes instead use  v = TS(b, cb, gamma); o = TT_add(t, v)  which folds
the root constant and writes the fp16 output tile directly (host casts to
fp32).  The split fractions below balance ACT/DVE/GPSIMD at ~118us/core.

DMA: leaf operands are host-pre-gathered into per-kernel-chunk contiguous
arrays (8 leaves x 844 positions per chunk) so inputs arrive in 32 large
~1.7MB DMAs instead of 128 small ones.
"""
import numpy as np

# ---- problem constants (hardcoded per contest contract) ----
B, C, H, W, D = 4, 3, 32, 32, 32
K, S = 32, 16
OH = OW = OD = 30
P = OH * OW * OD            # 27000
BP = B * P                  # 108000
NPART = 128
FREE = (BP + NPART - 1) // NPART   # 844
PADBP = NPART * FREE        # 108032
NCORES = 8
KLOC = K // NCORES          # 4
TEMP = 1.0
NLEV = 5
NODES_PER_K = 31            # 16+8+4+2+1
CHUNK = 4                   # leaves per input DMA chunk
NCHUNK = S // CHUNK         # 4 per kernel per operand
CFREE = CHUNK * FREE        # 3376
NCOLS = KLOC * (30 * 2 + 4)  # 256 coef cols (30 non-root x2 [s1,s2] + root x4)
LEV_OFF = [0, 32, 48, 56, 60]  # per-kernel coef column offset by level
SIG_CAP = 8192.0            # scale-gauge clamp (keeps fp16 tiles in range)

GATES = np.array([[(g >> t) & 1 for t in range(4)] for g in range(16)],
                 dtype=np.float64)

# engine assignment knobs.  GPSIMD is net-negative (concurrent GPS activity
# slows DVE ops ~3.5x), so everything runs on DVE+ACT: all tensor_scalar
# (u/v) ops on ACT (scalar engine), all tensor_tensor (t-mult, o-add) on DVE.
# Scale-gauge: each non-root node emits o' = sigma*o with sigma chosen so
# the o-op is a PURE tensor add (o' = b_tile + t'), eliminating the slow
# scalar_tensor_tensor op; host folds all scales into the u-op scalars.
TS_DVE_RES = ()          # TS op -> DVE when ts_idx % TS_DVE_MODB in RES
TS_DVE_MODB = 3
USE_GPS = False


# ----------------------------------------------------------------- host math
def _lut_coeffs(w):
    """w: (nodes,K,16) -> c0, ca, cb, cab each (nodes,K) float64."""
    w = w.astype(np.float64)
    e = np.exp((w - w.max(-1, keepdims=True)) / TEMP)
    p = e / e.sum(-1, keepdims=True)
    l = p @ GATES
    l0, l1, l2, l3 = l[..., 0], l[..., 1], l[..., 2], l[..., 3]
    return l0, l2 - l0, l1 - l0, l0 - l1 - l2 + l3


def _fold_coeffs(ws):
    """Fold per-node constants into parents.  Returns (folded, root_const):
    folded[lev] = (ca2, cb2, cab) each (nodes,K); root_const (K,)."""
    folded = []
    gamma = None
    for lev, w in enumerate(ws):
        c0, ca, cb, cab = _lut_coeffs(w)
        if lev == 0:
            gA = np.zeros_like(c0)
            gB = np.zeros_like(c0)
        else:
            gA = gamma[0::2]
            gB = gamma[1::2]
        folded.append((ca + cab * gB, cb + cab * gA, cab))
        gamma = c0 + ca * gA + cb * gB + cab * gA * gB
    return folded, gamma[0]


def _coef_cols(k, folded, root_const):
    """Per-kernel coef column values, in (level, index) order.

    Scale-gauge: node (lev,i) emits o' = sigma*o.  sigma(leaf) = 1/cb2;
    sigma(lev,i) = sigma(lev-1, 2i+1)/cb2, clamped to +-SIG_CAP, so that
    o' = b_tile + t' is a pure add.  u-op scalars absorb everything:
    s1 = cab*sig/(sigA*sigB), s2 = ca2*sig/sigA.  Root emits the true value:
    s1 = cab/(sigA*sigB), s2 = ca2/sigA, v-op = (cb2/sigB)*b + root_const."""
    sig = {}
    cols = []
    for lev in range(NLEV - 1):
        ca2, cb2, cab = folded[lev]
        for i in range(ca2.shape[0]):
            if lev == 0:
                sA = sB = 1.0
            else:
                sA = sig[(lev - 1, 2 * i)]
                sB = sig[(lev - 1, 2 * i + 1)]
            sg = float(np.clip(sB / cb2[i, k], -SIG_CAP, SIG_CAP))
            sig[(lev, i)] = sg
            cols += [cab[i, k] * sg / (sA * sB), ca2[i, k] * sg / sA]
    ca2, cb2, cab = folded[NLEV - 1]
    sA = sig[(NLEV - 2, 0)]
    sB = sig[(NLEV - 2, 1)]
    cols += [cab[0, k] / (sA * sB), ca2[0, k] / sA,
             cb2[0, k] / sB, root_const[k]]
    return cols


def _prep_inputs(x, kc, ws):
    """Build per-core in_maps (numpy)."""
    # 81 shifted windows, flattened positions (b,oh,ow,od), fp16, padded
    X81 = np.empty((3, 3, 3, 3, B, OH, OW, OD), np.float32)
    for c in range(3):
        for dh in range(3):
            for dw in range(3):
                for dd in range(3):
                    X81[c, dh, dw, dd] = x[:, c, dh:dh + 30, dw:dw + 30, dd:dd + 30]
    X81f = np.zeros((81, NPART, FREE), np.float16)
    X81f.reshape(81, PADBP)[:, :BP] = X81.reshape(81, BP).astype(np.float16)

    h_, w_, d_, c_ = kc[..., 0], kc[..., 1], kc[..., 2], kc[..., 3]
    sl = ((c_ * 3 + h_) * 3 + w_) * 3 + d_          # (2,K,S)

    folded, root_const = _fold_coeffs(ws)

    in_maps = []
    for core in range(NCORES):
        ks = range(core * KLOC, (core + 1) * KLOC)
        a_in = np.empty((KLOC * NCHUNK, NPART, CFREE), np.float16)
        b_in = np.empty((KLOC * NCHUNK, NPART, CFREE), np.float16)
        colv = []
        for kk, k in enumerate(ks):
            for c in range(NCHUNK):
                idx0 = sl[0, k, c * CHUNK:(c + 1) * CHUNK]
                idx1 = sl[1, k, c * CHUNK:(c + 1) * CHUNK]
                a_in[kk * NCHUNK + c] = \
                    X81f[idx0].transpose(1, 0, 2).reshape(NPART, CFREE)
                b_in[kk * NCHUNK + c] = \
                    X81f[idx1].transpose(1, 0, 2).reshape(NPART, CFREE)
            colv += _coef_cols(k, folded, root_const)
        assert len(colv) == NCOLS
        coef = np.broadcast_to(
            np.asarray(colv, np.float32), (NPART, NCOLS)).copy()
        in_maps.append({"a_in": a_in, "b_in": b_in, "coef": coef})
    return in_maps


# ------------------------------------------------------------ device program
def _build_program():
    import concourse.bass as bass
    import concourse.bacc as bacc
    import concourse.mybir as mybir
    from concourse.tile import TileContext

    f16 = mybir.dt.float16
    f32 = mybir.dt.float32
    Alu = mybir.AluOpType
    Act = mybir.ActivationFunctionType

    nc = bacc.Bacc()
    a_in = nc.declare_dram_parameter("a_in", [KLOC * NCHUNK, NPART, CFREE],
                                     f16, isOutput=False)
    b_in = nc.declare_dram_parameter("b_in", [KLOC * NCHUNK, NPART, CFREE],
                                     f16, isOutput=False)
    coef = nc.declare_dram_parameter("coef", [NPART, NCOLS], f32,
                                     isOutput=False)
    out = nc.declare_dram_parameter("out", [KLOC, NPART, FREE], f16,
                                    isOutput=True)

    ts_idx = 0
    o_idx = 0

    def eager_nodes():
        """Post-order (eager-reduction) node sequence for one kernel."""
        seq = []
        for s in range(S):
            seq.append((0, s))
            l, i = 0, s
            while i % 2 == 1 and l < NLEV - 1:
                l, i = l + 1, i // 2
                seq.append((l, i))
        return seq

    with TileContext(nc) as tc:
        with (
            tc.tile_pool(name="cpool", bufs=1) as cpool,
            tc.tile_pool(name="apool", bufs=9) as apool,
            tc.tile_pool(name="bpool", bufs=9) as bpool,
            tc.tile_pool(name="upool", bufs=5) as upool,
            tc.tile_pool(name="vpool", bufs=5) as vpool,
            tc.tile_pool(name="tpool", bufs=5) as tpool,
            tc.tile_pool(name="lpool", bufs=2) as lpool,
            tc.tile_pool(name="opool", bufs=3) as opool,
        ):
            coef_sb = cpool.tile([NPART, NCOLS], f32)
            nc.sync.dma_start(out=coef_sb[:], in_=coef[:])

            def ts_op(dst, src, scale_ap, bias_ap):
                nonlocal ts_idx
                if ts_idx % TS_DVE_MODB in TS_DVE_RES:
                    if bias_ap is None:
                        nc.vector.tensor_scalar(dst, src, scale_ap, None,
                                                Alu.mult)
                    else:
                        nc.vector.tensor_scalar(dst, src, scale_ap, bias_ap,
                                                Alu.mult, Alu.add)
                else:
                    nc.scalar.activation(dst, src, Act.Identity,
                                         bias=bias_ap if bias_ap is not None
                                         else 0.0,
                                         scale=scale_ap)
                ts_idx += 1

            # per-(kernel, lev, idx) state
            achunk = {}
            bchunk = {}
            otile = {}
            state = {}

            def col_of(kk, lev, i):
                return kk * 64 + LEV_OFF[lev] + (4 if lev == NLEV - 1
                                                 else 2) * i

            def inputs(kk, lev, i):
                if lev == 0:
                    c, j = i // CHUNK, i % CHUNK
                    a_ap = achunk[kk, c][:, j * FREE:(j + 1) * FREE]
                    b_ap = bchunk[kk, c][:, j * FREE:(j + 1) * FREE]
                    return a_ap, b_ap
                return (otile[kk, lev - 1, 2 * i][:],
                        otile[kk, lev - 1, 2 * i + 1][:])

            def stage_ts(kk, lev, i):
                col = col_of(kk, lev, i)
                a_ap, b_ap = inputs(kk, lev, i)
                is_root = lev == NLEV - 1
                u = upool.tile([NPART, FREE], f16, tag="u",
                               name=f"u{kk}_{lev}_{i}")
                ts_op(u[:], b_ap, coef_sb[:, col:col + 1],
                      coef_sb[:, col + 1:col + 2])
                v = None
                if is_root:
                    v = vpool.tile([NPART, FREE], f16, tag="v",
                                   name=f"v{kk}_{lev}_{i}")
                    ts_op(v[:], b_ap, coef_sb[:, col + 2:col + 3],
                          coef_sb[:, col + 3:col + 4])
                state[kk, lev, i] = (u, v, a_ap, b_ap)

            def stage_tt(kk, lev, i):
                u, v, a_ap, b_ap = state[kk, lev, i]
                t = tpool.tile([NPART, FREE], f16, tag="t",
                               name=f"t{kk}_{lev}_{i}")
                nc.vector.tensor_tensor(out=t[:], in0=a_ap, in1=u[:],
                                        op=Alu.mult)
                state[kk, lev, i] = (t, v, a_ap, b_ap)

            def stage_o(kk, lev, i):
                t, v, a_ap, b_ap = state.pop((kk, lev, i))
                is_root = lev == NLEV - 1
                if is_root:
                    ot = opool.tile([NPART, FREE], f16, tag="out",
                                    name=f"ot{kk}")
                    nc.vector.tensor_tensor(out=ot[:], in0=t[:], in1=v[:],
                                            op=Alu.add)
                    nc.sync.dma_start(out=out[kk], in_=ot[:])
                    return
                o = lpool.tile([NPART, FREE], f16, tag=f"o{lev}",
                               name=f"o{kk}_{lev}_{i}",
                               bufs=8 if lev == 0 else 6)
                nc.vector.tensor_tensor(out=o[:], in0=b_ap, in1=t[:],
                                        op=Alu.add)
                otile[kk, lev, i] = o

            for pair in range(KLOC // 2):
                kA, kB = 2 * pair, 2 * pair + 1
                for c in range(NCHUNK):
                    for kk in (kA, kB):
                        at = apool.tile([NPART, CFREE], f16, tag="a",
                                        name=f"a{kk}_{c}")
                        nc.sync.dma_start(out=at[:],
                                          in_=a_in[kk * NCHUNK + c])
                        achunk[kk, c] = at
                        bt = bpool.tile([NPART, CFREE], f16, tag="b",
                                        name=f"b{kk}_{c}")
                        nc.sync.dma_start(out=bt[:],
                                          in_=b_in[kk * NCHUNK + c])
                        bchunk[kk, c] = bt
                # interleave the two kernels' eager node streams
                nodes = []
                for na, nb in zip(eager_nodes(), eager_nodes()):
                    nodes.append((kA,) + na)
                    nodes.append((kB,) + nb)
                # software-pipelined emission: O(j-2), TT(j-1), TS(j)
                n = len(nodes)
                for j in range(n + 2):
                    if j >= 2:
                        stage_o(*nodes[j - 2])
                    if 1 <= j <= n:
                        stage_tt(*nodes[j - 1])
                    if j < n:
                        stage_ts(*nodes[j])
    nc.compile()
    return nc


_PROGRAM = None


def kernel(**inputs):
    global _PROGRAM
    x = np.asarray(inputs["x"], dtype=np.float32)
    kc = np.asarray(inputs["kernel_coords"])
    ws = [np.asarray(inputs[f"w{i}"]) for i in range(5)]

    in_maps = _prep_inputs(x, kc, ws)

    from concourse.bass_utils import run_bass_kernel_spmd
    if _PROGRAM is None:
        _PROGRAM = _build_program()
    res = run_bass_kernel_spmd(_PROGRAM, in_maps, list(range(NCORES)))
    results = res.results

    full = np.empty((K, PADBP), np.float32)
    for core in range(NCORES):
        o = results[core]["out"].reshape(KLOC, PADBP)
        full[core * KLOC:(core + 1) * KLOC] = o
    out = full[:, :BP].reshape(K, B, OH, OW, OD).transpose(1, 0, 2, 3, 4)
    return np.ascontiguousarray(out)
